# revision 1
# baseline (speedup 1.0000x reference)
"""Transformer-XL relative attention (B=2, L=2048, D=1024, H=16) on 8 TRN2
NeuronCores.

Sharding: data-parallel over batch x tensor-parallel over heads.  Core
c = 4*b + g handles batch b, head group g (4 heads).  Wq/Wk/Wv are
column-sharded, Wo row-sharded; each core emits a partial [2048,1024]
output which the host sums per batch (+bo).

Per-core layouts (bf16 in SBUF, head pair = h//2, row = 64*(h%2)+e):
  q1t/q2t/kt : [128, 2*2048]  e-tile h//2 at cols [2048*(h//2)], free = seq
  peht       : [128, 2*2048]  relative-position keys pe[1:2049] @ r_kernel
  vh         : [128, 16*260]  per key-tile: 4 heads' V (64 cols each) + a
                              ones column per head (free row-sum trick)
  at (A^T)   : [128, 2*2048]  normalized attention output, transposed

Rel-shift: for query block bi (rows ri..ri+127) R_s[rr, x] holds
Q2 . peh[xmin + x] (xmin = 1920 - ri); the score chunk at columns
[c0, c0+wc) needs staged[rr, cc] = R_s[rr, 127 - rr + c0 + cc] -- an
anti-diagonal flat access pattern (offset 127+c0, ap=[[PITCH-1,128],[1,wc]])
which only DMA engines can execute (SBUF->SBUF).  R_s columns beyond the
causal edge are padded with -1e9; the anti-diagonal read maps exactly the
strict upper triangle onto that pad, so exp() yields the causal zeros with
no separate masking pass.
"""

import numpy as np
import ml_dtypes
import concourse.bass as bass
import concourse.mybir as mybir
import concourse.tile as tile
from concourse import bacc
from concourse import bass_utils
from concourse.bass_interp import get_hw_module
from concourse.masks import make_identity

BF = mybir.dt.bfloat16
F32 = mybir.dt.float32
EXP = mybir.ActivationFunctionType.Exp
IDENT = mybir.ActivationFunctionType.Identity
MULT = mybir.AluOpType.mult
ADD = mybir.AluOpType.add

L = 2048          # sequence length
RS_W = 2176       # R_s tile width (2048 + 128 pad); anti-diag pitch = RS_W-1

PROFILE = False       # set by test harness to capture a trace
LAST_RESULTS = None   # BassKernelResults of the last run (for profiling)

# tuning knobs (read at build time)
CFG = {
    "diag_eng": "sync",    # engine issuing anti-diagonal staging DMAs
    "load_eng": "sync",    # engine issuing input/weight loads
    "rs_bufs": 3,
    "stg_bufs": 3,
    "pn_bufs": 5,
    "pt_bufs": 1,
    "psS_bufs": 2,
    "psR_bufs": 3,
    "psT_bufs": 2,
    "psA_bufs": 1,
    "osb_bufs": 2,
    "rec64_bufs": 1,
    "r_evict": "mix",     # act | dve | mix
    "pt_evict": "dve",    # act | dve | mix
    "io_bufs": 2,
    "proj_order": "kpvq",
    "hg_order": "gh",
    "rel_add": "pe",
    "pt_mode": "pe",   # pe | xbar
    "bcast": "gpsimd",  # gpsimd | dma
    "diag_split": 0,    # 0 = one staging DMA per row-block, 1 = per chunk
}


def emit_core(nc, ins, out):
    """ins: dict name->AP (DRAM), out: AP (DRAM [2048,1024] f32)."""
    with tile.TileContext(nc) as tc:
        deng = getattr(nc, CFG["diag_eng"])
        leng = getattr(nc, CFG["load_eng"])
        with (
            tc.tile_pool(name="wgt", bufs=1) as wp,
            tc.tile_pool(name="io", bufs=CFG["io_bufs"]) as iop,
            tc.tile_pool(name="per", bufs=1) as per,
            tc.tile_pool(name="work", bufs=3) as wk,
            tc.tile_pool(name="pt", bufs=CFG["pt_bufs"]) as ptp,
            tc.tile_pool(name="psS", bufs=CFG["psS_bufs"], space="PSUM") as psS,
            tc.tile_pool(name="psR", bufs=CFG["psR_bufs"], space="PSUM") as psR,
            tc.tile_pool(name="psA", bufs=CFG["psA_bufs"], space="PSUM") as psA,
            tc.tile_pool(name="psT", bufs=CFG["psT_bufs"], space="PSUM") as psT,
        ):
            # ---------------- persistent tiles ----------------
            q1t = per.tile([128, 2 * L], BF, tag="q1t")
            q2t = per.tile([128, 2 * L], BF, tag="q2t")
            kt = per.tile([128, 2 * L], BF, tag="kt")
            peht = per.tile([128, 2 * L], BF, tag="peht")
            at = per.tile([128, 2 * L], BF, tag="at")
            vh = per.tile([128, 16 * 260], BF, tag="vh")
            b1s = per.tile([128, 2], F32, tag="b1s")
            b2s = per.tile([128, 2], F32, tag="b2s")
            bks = per.tile([128, 2], F32, tag="bks")
            bvs = per.tile([128, 2], F32, tag="bvs")

            ident = per.tile([128, 128], BF, tag="ident")
            make_identity(nc, ident[:])
            nc.sync.dma_start(b1s[:], ins["b1"])
            nc.sync.dma_start(b2s[:], ins["b2"])
            nc.sync.dma_start(bks[:], ins["bk"])
            nc.sync.dma_start(bvs[:], ins["bv"])

            # ---------------- weights ----------------
            def load_w(name, cols):
                t = wp.tile([128, cols], BF, tag=name, name=f"w_{name}")
                leng.dma_start(
                    t[:], ins[name].rearrange("(a p) e -> p a e", p=128)
                )
                return t

            wq_s = load_w("wq", 2048)
            wk_s = load_w("wk", 2048)
            wv_s = load_w("wv", 2048)
            rk_s = load_w("rk", 2048)
            wo_s = load_w("wo", 2048)

            # ---------------- phase A: projections ----------------
            vht = per.tile([128, 2 * L], BF, tag="vht")

            def project(xname, w_s, evict):
                xt = iop.tile([128, 8 * L], BF, tag="inT", name=f"in_{xname}")
                leng.dma_start(
                    xt[:], ins[xname].rearrange("(a p) n -> p a n", p=128)
                )
                for et in range(2):
                    for rc in range(4):
                        ps = psS.tile([128, 512], F32, tag="S",
                                      name=f"ps_{xname}_{et}_{rc}")
                        for kc in range(8):
                            nc.tensor.matmul(
                                ps[:],
                                w_s[:, 256 * kc + 128 * et: 256 * kc + 128 * et + 128],
                                xt[:, 2048 * kc + 512 * rc: 2048 * kc + 512 * rc + 512],
                                start=(kc == 0),
                                stop=(kc == 7),
                            )
                        evict(et, rc, ps)

            def evict_q(et, rc, ps):
                cs = 2048 * et + 512 * rc
                nc.scalar.activation(q1t[:, cs:cs + 512], ps[:], IDENT,
                                     bias=b1s[:, et:et + 1], scale=0.125)
                nc.scalar.activation(q2t[:, cs:cs + 512], ps[:], IDENT,
                                     bias=b2s[:, et:et + 1], scale=0.125)

            def evict_k(et, rc, ps):
                cs = 2048 * et + 512 * rc
                nc.scalar.activation(kt[:, cs:cs + 512], ps[:], IDENT,
                                     bias=bks[:, et:et + 1], scale=1.0)

            def evict_v(et, rc, ps):
                cs = 2048 * et + 512 * rc
                nc.scalar.activation(vht[:, cs:cs + 512], ps[:], IDENT,
                                     bias=bvs[:, et:et + 1], scale=1.0)

            def evict_pe(et, rc, ps):
                cs = 2048 * et + 512 * rc
                nc.vector.tensor_copy(peht[:, cs:cs + 512], ps[:])

            projs = {"q": ("qT", wq_s, evict_q), "k": ("kT", wk_s, evict_k),
                     "v": ("vT", wv_s, evict_v), "p": ("peT", rk_s, evict_pe)}
            for c in CFG["proj_order"]:
                project(*projs[c])

            # vh assembly: PE-transpose one [128,128] tile per (ct, pair)
            # (covers both heads of the pair), evict the two heads' column
            # halves into their vh blocks.
            for ct in range(16):
                for p in range(2):
                    scr = psT.tile([128, 512], BF, tag="T",
                                   name=f"scr_{ct}_{p}")
                    nc.tensor.transpose(
                        scr[:, 0:128],
                        vht[:, 2048 * p + 128 * ct: 2048 * p + 128 * ct + 128],
                        ident[:],
                    )
                    for hh in range(2):
                        h4 = 2 * p + hh
                        nc.vector.tensor_copy(
                            vh[:, 260 * ct + 65 * h4: 260 * ct + 65 * h4 + 64],
                            scr[:, 64 * hh: 64 * hh + 64],
                        )
            ones_cols = vh[:].rearrange("p (ct c) -> p ct c", c=260)
            for h in range(4):
                nc.vector.memset(ones_cols[:, :, 65 * h + 64], 1.0)

            def outproj_rt(rt):
                osb = wk.tile([128, 1024], F32, tag="osb",
                              bufs=CFG["osb_bufs"], name=f"osb_{rt}")
                for n in range(2):
                    op_ = psS.tile([128, 512], F32, tag="S",
                                   name=f"op_{rt}_{n}")
                    for hc in range(2):
                        nc.tensor.matmul(
                            op_[:],
                            at[:, 2048 * hc + 128 * rt: 2048 * hc + 128 * rt + 128],
                            wo_s[:, 1024 * hc + 512 * n: 1024 * hc + 512 * n + 512],
                            start=(hc == 0), stop=(hc == 1),
                        )
                    if n == 0:
                        nc.scalar.copy(osb[:, 0:512], op_[:])
                    else:
                        nc.vector.tensor_copy(osb[:, 512:1024], op_[:])
                nc.sync.dma_start(out[128 * rt: 128 * rt + 128, :], osb[:])

            # ---------------- phase B: attention ----------------
            hg_order = ([(h, g) for h in range(4) for g in range(4)]
                        if CFG["hg_order"] == "hg" else
                        [(h, g) for g in range(4) for h in range(4)])
            for h, g in hg_order:
                et, ph = h // 2, h % 2
                r0, r1 = 64 * ph, 64 * ph + 64
                if True:
                    # P^T for the whole (head, group): strip ci lives at
                    # columns [512*ci, 512*ci+512) (q within the group).
                    pt = ptp.tile([128, 512 * (4 * g + 4)], BF, tag="pt",
                                  bufs=CFG["pt_bufs"], name=f"pt_h{h}_g{g}")
                    for bi in range(4 * g, 4 * g + 4):
                        ri = 128 * bi
                        Wb = ri + 128
                        nch = (Wb + 511) // 512
                        xmin = 1920 - ri   # peht col of R_s col 0
                        rs = wk.tile([128, RS_W], BF, tag="rs",
                                     bufs=CFG["rs_bufs"],
                                     name=f"rs_h{h}_b{bi}")
                        # rel matmuls + evict to rs (bf16)
                        for jc in range(nch):
                            w = min(512, Wb - 512 * jc)
                            rp = psR.tile([128, 512], F32, tag="R",
                                          name=f"rp_h{h}_b{bi}_{jc}")
                            nc.tensor.matmul(
                                rp[:, :w],
                                q2t[r0:r1, 2048 * et + ri: 2048 * et + ri + 128],
                                peht[r0:r1, 2048 * et + xmin + 512 * jc:
                                     2048 * et + xmin + 512 * jc + w],
                                start=True, stop=True,
                            )
                            use_act = (CFG["r_evict"] == "act" or
                                       (CFG["r_evict"] == "mix" and jc % 2 == 0))
                            if use_act:
                                nc.scalar.copy(rs[:, 512 * jc: 512 * jc + w],
                                               rp[:, :w])
                            else:
                                nc.vector.tensor_copy(
                                    rs[:, 512 * jc: 512 * jc + w], rp[:, :w])
                        nc.vector.memset(rs[:, Wb:Wb + 128], -1e9)
                        # anti-diagonal staging: one DMA per row-block, or
                        # per 512-chunk (shorter dependency chain)
                        staged = wk.tile([128, 2048], BF, tag="stg",
                                         bufs=CFG["stg_bufs"],
                                         name=f"stg_h{h}_b{bi}")
                        if CFG["diag_split"]:
                            for dc in range(nch):
                                d0 = 512 * dc
                                dw = min(512, Wb - d0)
                                diag = bass.AP(
                                    tensor=rs.tensor,
                                    offset=rs.offset + 127 + d0,
                                    ap=[[RS_W - 1, 128], [1, dw]],
                                )
                                deng.dma_start(staged[:, d0:d0 + dw], diag)
                        else:
                            diag = bass.AP(
                                tensor=rs.tensor,
                                offset=rs.offset + 127,
                                ap=[[RS_W - 1, 128], [1, Wb]],
                            )
                            deng.dma_start(staged[:, :Wb], diag)
                        # content scores + rel add + exp + PE transpose
                        for ci_chunk in range(nch):
                            c0 = 512 * ci_chunk
                            wc = min(512, Wb - c0)
                            sp = psS.tile([128, 512], F32, tag="S",
                                          name=f"sp_h{h}_b{bi}_{ci_chunk}")
                            nc.tensor.matmul(
                                sp[:, :wc],
                                q1t[r0:r1, 2048 * et + ri: 2048 * et + ri + 128],
                                kt[r0:r1, 2048 * et + c0: 2048 * et + c0 + wc],
                                start=True,
                                stop=not (CFG["rel_add"] == "pe" or
                                          (CFG["rel_add"] == "mix"
                                           and (bi + ci_chunk) % 2 == 0)),
                            )
                            # rel-shift add: identity-matmul accumulate (PE)
                            # or a DVE scalar_tensor_tensor pass
                            use_pe = (CFG["rel_add"] == "pe" or
                                      (CFG["rel_add"] == "mix"
                                       and (bi + ci_chunk) % 2 == 0))
                            if use_pe:
                                nc.tensor.matmul(
                                    sp[:, :wc],
                                    ident[:],
                                    staged[:, c0:c0 + wc],
                                    start=False, stop=True,
                                )
                            else:
                                nc.vector.scalar_tensor_tensor(
                                    out=sp[:, :wc], in0=sp[:, :wc], scalar=1.0,
                                    in1=staged[:, c0:c0 + wc],
                                    op0=MULT, op1=ADD,
                                )
                            pn = wk.tile([128, 512], BF, tag="pn",
                                         bufs=CFG["pn_bufs"],
                                         name=f"pn_h{h}_b{bi}_{ci_chunk}")
                            nc.scalar.activation(pn[:, :wc], sp[:, :wc], EXP)
                            if CFG["pt_mode"] == "xbar":
                                for s in range(wc // 128):
                                    ci = c0 // 128 + s
                                    nc.sync.dma_start_transpose(
                                        pt[:, 512 * ci + 128 * (bi % 4):
                                           512 * ci + 128 * (bi % 4) + 128],
                                        pn[:, 128 * s: 128 * s + 128],
                                    )
                                continue
                            tp_ = psT.tile([128, 512], BF, tag="T",
                                           name=f"tp_h{h}_b{bi}_{ci_chunk}")
                            for s in range(wc // 128):
                                nc.tensor.transpose(
                                    tp_[:, 128 * s: 128 * s + 128],
                                    pn[:, 128 * s: 128 * s + 128],
                                    ident[:],
                                )
                            dst = bass.AP(
                                tensor=pt.tensor,
                                offset=pt.offset + 512 * (c0 // 128)
                                + 128 * (bi % 4),
                                ap=[[512 * (4 * g + 4), 128],
                                    [512, wc // 128], [1, 128]],
                            )
                            use_act = (CFG["pt_evict"] == "act" or
                                       (CFG["pt_evict"] == "mix"
                                        and (bi + ci_chunk) % 2 == 0))
                            if use_act:
                                nc.scalar.copy(dst, tp_[:, :wc])
                            else:
                                nc.vector.tensor_copy(dst, tp_[:, :wc])
                    # AV for this (head, group); row 64 = softmax denominators
                    av = psA.tile([65, 512], F32, tag="A",
                                  name=f"av_h{h}_g{g}")
                    for ci in range(4 * g + 4):
                        o = max(0, 128 * ci - 512 * g)
                        nc.tensor.matmul(
                            av[:, o:512],
                            vh[:, 260 * ci + 65 * h: 260 * ci + 65 * h + 65],
                            pt[:, 512 * ci + o: 512 * ci + 512],
                            start=(ci == 0), stop=(ci == 4 * g + 3),
                        )
                    rec = wk.tile([1, 512], F32, tag="rec",
                                  name=f"rec_h{h}_g{g}")
                    nc.vector.reciprocal(rec[:], av[64:65, :])
                    rec64 = wk.tile([64, 512], F32, tag="rec64", bufs=CFG["rec64_bufs"],
                                    name=f"rec64_h{h}_g{g}")
                    nc.gpsimd.partition_broadcast(rec64[:], rec[:])
                    nc.vector.tensor_tensor(
                        out=at[r0:r1, 2048 * et + 512 * g:
                               2048 * et + 512 * g + 512],
                        in0=av[0:64, :],
                        in1=rec64[:],
                        op=MULT,
                    )
                    if CFG["hg_order"] == "gh" and h == 3:
                        for rt in range(4 * g, 4 * g + 4):
                            outproj_rt(rt)

            # ---------------- phase C: output projection ----------------
            if CFG["hg_order"] != "gh":
                for rt in range(16):
                    outproj_rt(rt)
    return nc


# ---------------- host side ----------------

def _bf16(x):
    return np.ascontiguousarray(x).astype(ml_dtypes.bfloat16)


def _col2d(vec256):
    """[256] f32 -> [128, 2] with v2d[p, a] = vec[128a + p]."""
    return np.ascontiguousarray(
        np.asarray(vec256, np.float32).reshape(2, 128).T)


def core_inputs(q_b, k_b, v_b, pos_enc, Wq, bq, Wk, bk, Wv, bv, Wo,
                r_w_bias, r_r_bias, r_kernel, g):
    sl = slice(256 * g, 256 * g + 256)
    rk_cat = np.concatenate([r_kernel[4 * g + i] for i in range(4)], axis=1)
    return {
        "qT": _bf16(q_b.T),
        "kT": _bf16(k_b.T),
        "vT": _bf16(v_b.T),
        "peT": _bf16(pos_enc[1:2049].T),
        "wq": _bf16(Wq[:, sl]),
        "wk": _bf16(Wk[:, sl]),
        "wv": _bf16(Wv[:, sl]),
        "rk": _bf16(rk_cat),
        "wo": _bf16(Wo[sl, :]),
        "b1": _col2d(0.125 * (bq[sl] + r_w_bias[4 * g:4 * g + 4].reshape(256))),
        "b2": _col2d(0.125 * (bq[sl] + r_r_bias[4 * g:4 * g + 4].reshape(256))),
        "bk": _col2d(bk[sl]),
        "bv": _col2d(bv[sl]),
    }


_SHAPES = {
    "qT": ([1024, 2048], BF), "kT": ([1024, 2048], BF),
    "vT": ([1024, 2048], BF), "peT": ([1024, 2048], BF),
    "wq": ([1024, 256], BF), "wk": ([1024, 256], BF), "wv": ([1024, 256], BF),
    "rk": ([1024, 256], BF), "wo": ([256, 1024], BF),
    "b1": ([128, 2], F32), "b2": ([128, 2], F32),
    "bk": ([128, 2], F32), "bv": ([128, 2], F32),
}

_NC_CACHE = {}


def _build():
    key = tuple(sorted(CFG.items()))
    if key in _NC_CACHE:
        return _NC_CACHE[key]
    nc = bacc.Bacc("TRN2", target_bir_lowering=False, debug=False,
                   enable_asserts=False)
    ins = {name: nc.dram_tensor(name, shape, dt, kind="ExternalInput").ap()
           for name, (shape, dt) in _SHAPES.items()}
    out = nc.dram_tensor("out", [2048, 1024], F32, kind="ExternalOutput").ap()
    emit_core(nc, ins, out)
    nc.compile()
    nc.m = get_hw_module(nc.m)
    _NC_CACHE[key] = nc
    return nc


def kernel(**inputs):
    global LAST_RESULTS
    inp = {k: np.asarray(v) for k, v in inputs.items()}
    nc = _build()
    in_maps = []
    for c in range(8):
        b, g = c // 4, c % 4
        in_maps.append(core_inputs(
            inp["q"][b], inp["k"][b], inp["v"][b], inp["pos_enc"],
            inp["Wq"], inp["bq"], inp["Wk"], inp["bk"], inp["Wv"], inp["bv"],
            inp["Wo"], inp["r_w_bias"], inp["r_r_bias"], inp["r_kernel"], g))
    res = bass_utils.run_bass_kernel_spmd(
        nc, in_maps, core_ids=list(range(8)), trace=PROFILE)
    LAST_RESULTS = res
    out = np.zeros((2, 2048, 1024), np.float32)
    for c in range(8):
        b = c // 4
        out[b] += res.results[c]["out"]
    out += np.asarray(inp["bo"], np.float32)[None, None, :]
    return out

